# revision 53
# baseline (speedup 1.0000x reference)
"""GAT (2-layer graph attention network) Trainium2 Bass kernel, 8-core SPMD.

Sharding (v2): every core computes ALL 8 layer-1 heads but only its own
512-column i-slice of the attention output (column-parallel), and the same
i-slice of layer 2. The adjacency slice [4096, 512] is loaded once per core
(two big DMAs) and reused by all 8 heads AND layer 2. Layer-1 -> layer-2
exchange is an AllGather of the per-core h @ Wo rows, split into a 384-row
and a 128-row piece so the first gather hides under the second piece's
attention loop.

Key math: exp(leaky_relu(s)) with s = f_src_i + f_dst_j factorizes as
e^{f_i} * max(g_i * r'_j, e1_j) with g_i = e^{(a-1) f_i},
r'_j = e^{a f_dst_j - C}, e1_j = e^{f_dst_j - C}. The e^{f_i} factor cancels
in the softmax, so the inner loop per (head, j-tile) is ONE
tensor_scalar (mult+max with two per-partition scalars, 4x DVE mode) and one
mask multiply (tensor_tensor, batched over 4 j-tiles), feeding a PE matmul
whose lhsT is the raw [Wh | 1] tile (ones column accumulates the softmax
denominator).

kernel(**inputs) takes full unsharded inputs, returns the full output.
"""

from contextlib import ExitStack

import numpy as np

import concourse.mybir as mybir
import concourse.tile as tile
from concourse import bacc
from concourse.bass_utils import run_bass_kernel_spmd
from concourse.masks import make_identity

# Steer every activation to the one ACT table set covering all functions this
# kernel uses (Exp, Copy, Identity) so no mid-kernel table reloads happen.
_orig_get_tables = bacc.get_activation_tables


def _pinned_tables(arch):
    tabs = _orig_get_tables(arch)
    if "natural_log_exp_and_others" in tabs:
        return {name: (funcs if name == "natural_log_exp_and_others" else set())
                for name, funcs in tabs.items()}
    return tabs


bacc.get_activation_tables = _pinned_tables

N = 4096
F = 512
D = 64          # per-head hidden == n classes
H = 8
P = 128
NT = N // P             # 32 j tiles
NKF = F // P            # 4 contraction tiles for x @ W
SLICE = N // 8          # 512 i columns per core
ALPHA = 0.2
AM1 = ALPHA - 1.0       # -0.8
C_DST = 1.0
PIECES = [(0, 512)]               # i-piece (offset, width) within the slice
N_CORES = 8
E = D + 2               # 66: [Wh | f_src | f_dst]

F32 = mybir.dt.float32
F16 = mybir.dt.float16

_CACHED = {}

AF = mybir.ActivationFunctionType
ALU = mybir.AluOpType


def build_kernel():
    nc = bacc.Bacc("TRN2", num_devices=N_CORES)

    xtr = nc.dram_tensor("xtr", [P, NKF * N], F16, kind="ExternalInput")
    xslr = nc.dram_tensor("xslr", [P, NKF * SLICE], F16, kind="ExternalInput")
    adjc = nc.dram_tensor("adjc", [P, NT * SLICE], F16, kind="ExternalInput")
    wextr = nc.dram_tensor("wextr", [P, NKF * H * E], F16,
                           kind="ExternalInput")
    woA = nc.dram_tensor("woA", [D, H * E], F16, kind="ExternalInput")
    outT = nc.dram_tensor("outT", [D, SLICE], F32, kind="ExternalOutput")

    with ExitStack() as ctx:
        tc = ctx.enter_context(tile.TileContext(nc))
        psum = ctx.enter_context(tc.tile_pool(name="psum", bufs=1, space="PSUM"))
        persist = ctx.enter_context(tc.tile_pool(name="persist", bufs=1))
        work = ctx.enter_context(tc.tile_pool(name="work", bufs=1))
        dram = ctx.enter_context(tc.tile_pool(name="dram", bufs=1, space="DRAM"))
        pools = {"psum": psum, "persist": persist, "work": work, "dram": dram}

        ident = persist.tile([P, P], F32, tag="ident")
        make_identity(nc, ident[:])
        ones_all = persist.tile([P, P], F32, tag="ones_all")
        nc.vector.memset(ones_all[:], 1.0)
        ones65 = persist.tile([D + 1, D], F32, tag="ones65")
        nc.vector.memset(ones65[:], 1.0)
        bias_d = persist.tile([P, 1], F32, tag="bias_d")
        nc.vector.memset(bias_d[:], -C_DST)
        _CACHED["bias_d"] = bias_d
        bias_p1 = persist.tile([P, 1], F32, tag="bias_p1")
        nc.vector.memset(bias_p1[:], 1.0)
        _CACHED["bias_p1"] = bias_p1
        _CACHED["ident"] = ident
        _CACHED["ones_all"] = ones_all
        _CACHED["ones65"] = ones65

        pools["tc"] = tc
        _emit(nc, pools, xtr, xslr, adjc, wextr, woA, outT)

    nc.compile()
    return nc


def _emit(nc, pools, xtr, xslr, adjc, wextr, woA, outT):
    psum, persist, work, dram = (pools["psum"], pools["persist"],
                                 pools["work"], pools["dram"])
    tc = pools["tc"]
    ident = _CACHED["ident"]
    ones_all = _CACHED["ones_all"]
    ones65 = _CACHED["ones65"]

    # ---- input DMAs (few, large) ----
    wextr_sb = persist.tile([P, NKF * H * E], F16, tag="wextr")
    nc.sync.dma_start(out=wextr_sb[:], in_=wextr[:])
    xt_sb = persist.tile([P, NKF * N], F16, tag="xt")
    NH = N // 2
    for half in range(2):
        nc.sync.dma_start(
            out=xt_sb[:].rearrange("p (kf n) -> p kf n", kf=NKF)[
                :, :, half * NH:(half + 1) * NH],
            in_=xtr[:].rearrange("p (kf n) -> p kf n", kf=NKF)[
                :, :, half * NH:(half + 1) * NH])
    xsl_sb = persist.tile([P, NKF * SLICE], F16, tag="xsl")
    nc.sync.dma_start(out=xsl_sb[:], in_=xslr[:])
    adj_sb = persist.tile([P, NT * SLICE], F16, tag="adj")
    QU = NT * SLICE // 4
    for q in range(4):
        nc.sync.dma_start(out=adj_sb[:, q * QU:(q + 1) * QU],
                          in_=adjc[:, q * QU:(q + 1) * QU])
    woA_sb = persist.tile([D, H * E], F16, tag="woA")
    nc.sync.dma_start(out=woA_sb[:], in_=woA[:])

    # ---- per-head prep: whs = [Wh | 1] fp16, e1 = exp(f_dst - C),
    #      r' = exp(a*f_dst - C), g_row = exp((a-1) f_src) broadcast ----
    whs, e1_sb, rp_sb, g_row = [], [], [], []
    for h in range(H):
        wb = persist.tile([P, NT * (D + 1)], F16, tag=f"whs{h}",
                          name=f"whs_{h}")
        nc.vector.memset(
            wb[:].rearrange("p (t c) -> p t c", t=NT)[:, :, D:D + 1], 1.0)
        e1 = persist.tile([P, NT], F32, tag=f"e1_{h}", name=f"e1_{h}")
        rp = persist.tile([P, NT], F32, tag=f"rp_{h}", name=f"rp_{h}")
        wb3 = wb[:].rearrange("p (t c) -> p t c", t=NT)
        fcol = work.tile([P, NT], F32, tag="fcol", bufs=2, name=f"fcol_{h}")
        fc3 = fcol[:].rearrange("p (k one) -> p k one", one=1)
        for grp in range(NT // 4):
            wh_ps = psum.tile([P, 4 * E], F32, tag="bank", bufs=3,
                              name=f"whps_{h}_{grp}")
            for k in range(4):
                t = grp * 4 + k
                for kf in range(NKF):
                    nc.tensor.matmul(
                        wh_ps[:, k * E:(k + 1) * E],
                        xt_sb[:, kf * N + t * P:kf * N + (t + 1) * P],
                        wextr_sb[:, (kf * H + h) * E:(kf * H + h + 1) * E],
                        start=(kf == 0), stop=(kf == NKF - 1))
            ps3 = wh_ps[:].rearrange("p (k c) -> p k c", k=4)
            # stage the f_dst column; exp per half-head so attention on the
            # first tiles can start before the whole head's prep finishes
            nc.scalar.activation(fc3[:, grp * 4:(grp + 1) * 4, :],
                                 ps3[:, :, D + 1:D + 2], AF.Copy)
            nc.scalar.activation(wb3[:, grp * 4:(grp + 1) * 4, 0:D],
                                 ps3[:, :, 0:D], AF.Copy)
            if grp == 3 or grp == 7:
                hsl = slice(0, 16) if grp == 3 else slice(16, 32)
                nc.scalar.activation(e1[:, hsl], fcol[:, hsl], AF.Exp,
                                     bias=_CACHED["bias_d"][:])
                nc.scalar.activation(rp[:, hsl], fcol[:, hsl], AF.Exp,
                                     bias=_CACHED["bias_d"][:], scale=ALPHA)
        whs.append(wb)
        e1_sb.append(e1)
        rp_sb.append(rp)

        fs_ps = psum.tile([1, SLICE], F32, tag="bank", bufs=3,
                          name=f"fsps_{h}")
        for kf in range(NKF):
            nc.tensor.matmul(
                fs_ps[:],
                wextr_sb[:, (kf * H + h) * E + D:(kf * H + h) * E + D + 1],
                xsl_sb[:, kf * SLICE:(kf + 1) * SLICE],
                start=(kf == 0), stop=(kf == NKF - 1))
        fs_sb = work.tile([1, SLICE], F32, tag="fs_sb", bufs=2,
                          name=f"fssb_{h}")
        nc.scalar.activation(fs_sb[:], fs_ps[:], AF.Copy)
        g_ps = psum.tile([P, SLICE], F32, tag="bank", bufs=3, name=f"gps_{h}")
        for ch in range(SLICE // P):
            nc.tensor.matmul(g_ps[:, ch * P:(ch + 1) * P],
                             ones_all[0:1, 0:P],
                             fs_sb[0:1, ch * P:(ch + 1) * P],
                             start=True, stop=True)
        gr = persist.tile([P, SLICE], F16, tag=f"g_{h}", name=f"g_{h}")
        nc.scalar.activation(gr[:], g_ps[:], AF.Exp, scale=AM1)
        g_row.append(gr)

    # ---- layer-1 attention + (pipelined) norm/ELU/p2, two i-pieces ----
    p2AB_ps = psum.tile([P, 4 * E], F32, tag="p2AB", name="p2AB")
    p2A_ps = p2AB_ps[:, 0:4 * E]
    cc_inA = dram.tile([4 * P, E], F16, tag="cc_inA", name="cc_inA")
    cc_fullA = dram.tile([N_CORES * 4 * P, E], F16, tag="cc_fullA",
                         addr_space="Shared", name="cc_fullA")

    ADJ_B0 = NT * PIECES[0][1]     # start col of the B block

    def adj_piece(pi, t, ntiles=1):
        pw = PIECES[pi][1]
        base = 0 if pi == 0 else ADJ_B0
        return adj_sb[:, base + t * pw:base + (t + ntiles) * pw]

    def attention(tag, pi, pw, g_ap, rp_ap, e1_ap, whs_ap, acc, hh=0):
        off = PIECES[pi][0]
        for grp in range(NT // 4):
            # offload a fraction of whole groups (TSPs + mask TT) to the
            # otherwise-idle GpSimd engine
            on_pool = ((hh * 8 + grp) % 5) == 2
            eng = nc.gpsimd if on_pool else nc.vector
            vg = work.tile([P, 4 * pw], F16, tag="vg", bufs=3,
                           name=f"vg_{tag}_{grp}")
            for k in range(4):
                t = grp * 4 + k
                eng.tensor_scalar(
                    vg[:, k * pw:(k + 1) * pw], g_ap[:, off:off + pw],
                    rp_ap[:, t:t + 1], e1_ap[:, t:t + 1], ALU.mult, ALU.max)
            wg = work.tile([P, 4 * pw], F16, tag="wg", bufs=4,
                           name=f"wg_{tag}_{grp}")
            eng.tensor_tensor(wg[:], vg[:], adj_piece(pi, grp * 4, 4),
                              ALU.mult)
            for k in range(4):
                t = grp * 4 + k
                nc.tensor.matmul(
                    acc[:], whs_ap[:, t * (D + 1):(t + 1) * (D + 1)],
                    wg[:, k * pw:(k + 1) * pw],
                    start=(t == 0), stop=(t == NT - 1))

    def elu(tag, x_ap, out_ap, pw):
        # elu(x) = max(x,0) - relu(1 - exp(x))
        e = work.tile([D, pw], F32, tag="elu_e", bufs=2, name=f"ele_{tag}")
        nc.scalar.activation(e[:], x_ap, AF.Exp)
        t1 = work.tile([D, pw], F32, tag="elu_t", bufs=2, name=f"elt_{tag}")
        nc.scalar.activation(t1[:], e[:], AF.Relu, scale=-1.0,
                             bias=_CACHED["bias_p1"][0:D, :])
        nc.vector.scalar_tensor_tensor(out_ap, x_ap, 0.0, t1[:],
                                       ALU.max, ALU.subtract)

    def norm_elu_p2(tag, acc, pw, h, fs2_ps):
        lnr = work.tile([1, pw], F32, tag="lnr", bufs=2, name=f"ln_{tag}")
        nc.scalar.activation(lnr[:], acc[D:D + 1, :], AF.Ln)
        rec = work.tile([1, pw], F32, tag="rec", bufs=2, name=f"rc_{tag}")
        nc.scalar.activation(rec[:], lnr[:], AF.Exp, scale=-1.0)
        den = psum.tile([D, pw], F32, tag="bank", bufs=3, name=f"den_{tag}")
        nc.tensor.matmul(den[:], ones_all[0:1, 0:D], rec[0:1, :],
                         start=True, stop=True)
        num = work.tile([D, pw], F32, tag="num", bufs=3, name=f"num_{tag}")
        nc.scalar.activation(num[:], acc[0:D, :], AF.Copy)
        x = work.tile([D, pw], F32, tag="xat", bufs=3, name=f"x_{tag}")
        nc.vector.tensor_tensor(x[:], num[:], den[:], ALU.mult)
        o_sb = work.tile([D, pw], F16, tag="osb", bufs=10, name=f"o_{tag}")
        elu(tag, x[:], o_sb[:], pw)
        # f_src2 row accumulates in its own bank; one open group there is
        # safe. The p2 chunk matmuls are emitted after the head loop so
        # each bank has at most one open accumulation group at a time.
        nc.tensor.matmul(
            fs2_ps, woA_sb[:, h * E + D:h * E + D + 1], o_sb[:],
            start=(h == 0), stop=(h == H - 1))
        return o_sb

    p2_sb = {}
    fs2_sb = work.tile([1, SLICE], F32, tag="fs2_sb", name="fs2_sb")
    fs2all_ps = psum.tile([1, SLICE], F32, tag="fs2", name="fs2")
    for pi, (off, pw) in enumerate(PIECES):
        nch = pw // P
        p2_ps = p2A_ps

        fs2_ps = fs2all_ps[0:1, off:off + pw]
        pending = None
        o_list = []
        for h in range(H):
            acc = psum.tile([D + 1, pw], F32, tag="acc", bufs=3,
                            name=f"acc_{pi}_{h}")
            attention(f"l1_{pi}_{h}", pi, pw, g_row[h][:], rp_sb[h][:],
                      e1_sb[h][:], whs[h][:], acc, hh=h)
            if pending is not None:
                o_list.append(norm_elu_p2(
                    f"l1_{pi}_{pending[0]}", pending[1], pw, pending[0],
                    fs2_ps))
            pending = (h, acc)
        with tc.high_priority():
            o_list.append(norm_elu_p2(f"l1_{pi}_{pending[0]}", pending[1],
                                      pw, pending[0], fs2_ps))
            for ch in range(nch):
                for h in range(H):
                    nc.tensor.matmul(
                        p2_ps[:, ch * E:(ch + 1) * E],
                        o_list[h][:, ch * P:(ch + 1) * P],
                        woA_sb[:, h * E:(h + 1) * E],
                        start=(h == 0), stop=(h == H - 1))
            nc.scalar.activation(fs2_sb[0:1, off:off + pw], fs2_ps, AF.Copy)
            ps = persist.tile([P, nch * E], F16, tag=f"p2sb_{pi}",
                              name=f"p2sb_{pi}")
            nc.scalar.activation(ps[:], p2_ps, AF.Copy)
            p2_sb[pi] = ps
            cc_in = cc_inA
            nc.sync.dma_start(
                out=cc_in[:].rearrange("(c p) d -> p c d", p=P),
                in_=ps[:].rearrange("p (c d) -> p c d", c=nch))

    nc.gpsimd.collective_compute(
        "AllGather", ALU.bypass, ins=[cc_inA[:]], outs=[cc_fullA[:]],
        replica_groups=[list(range(N_CORES))])

    # ---- g2_row from this core's own f_src2 row ----
    g2_ps = psum.tile([P, SLICE], F32, tag="bank", bufs=3, name="g2ps")
    for ch in range(SLICE // P):
        nc.tensor.matmul(g2_ps[:, ch * P:(ch + 1) * P],
                         ones_all[0:1, 0:P], fs2_sb[0:1, ch * P:(ch + 1) * P],
                         start=True, stop=True)
    g2_row = persist.tile([P, SLICE], F16, tag="g2", name="g2")
    nc.scalar.activation(g2_row[:], g2_ps[:], AF.Exp, scale=AM1)

    # ---- layer-2 prep from gathered rows ----
    whs2 = persist.tile([P, NT * (D + 1)], F16, tag="whs2", name="whs2")
    nc.vector.memset(
        whs2[:].rearrange("p (t c) -> p t c", t=NT)[:, :, D:D + 1], 1.0)
    e1_2 = persist.tile([P, NT], F32, tag="e1_2", name="e1_2")
    rp_2 = persist.tile([P, NT], F32, tag="rp_2", name="rp_2")

    ccA_sb = persist.tile([P, NT * E], F16, tag="ccA", name="ccA")
    NU = NT // 2
    for uh in range(2):
        nc.sync.dma_start(
            out=ccA_sb[:].rearrange("p (u d) -> p u d", u=NT)[
                :, uh * NU:(uh + 1) * NU, :],
            in_=cc_fullA[:].rearrange("(u p) d -> p u d", p=P)[
                :, uh * NU:(uh + 1) * NU, :])

    w23 = whs2[:].rearrange("p (t c) -> p t c", t=NT)

    def u_of_t(t):
        return t

    def l2_prep(cc_sb, n_u, u0, g_of_u):
        cc3 = cc_sb[:].rearrange("p (u c) -> p u c", u=NT)[
            :, u0:u0 + n_u, :]
        fc2 = work.tile([P, n_u], F32, tag="fc2", bufs=2, name=f"fc2_{u0}")
        nc.scalar.activation(
            fc2[:].rearrange("p (u one) -> p u one", one=1),
            cc3[:, :, D + 1:D + 2], AF.Copy)
        nc.scalar.activation(e1_2[:, u0:u0 + n_u], fc2[:], AF.Exp,
                             bias=_CACHED["bias_d"][:])
        nc.scalar.activation(rp_2[:, u0:u0 + n_u], fc2[:], AF.Exp,
                             bias=_CACHED["bias_d"][:], scale=ALPHA)
        for uu in range(n_u):
            u = u0 + uu
            if u % 2 == 0:
                nc.scalar.activation(w23[:, g_of_u(u), 0:D],
                                     cc3[:, uu, 0:D], AF.Copy)
            else:
                nc.gpsimd.tensor_copy(w23[:, g_of_u(u), 0:D],
                                      cc3[:, uu, 0:D])

    l2_prep(ccA_sb, NT // 2, 0, lambda u: u)
    l2_prep(ccA_sb, NT // 2, NT // 2, lambda u: u)

    # ---- layer-2 attention, same two i-pieces as layer 1 ----
    fin = persist.tile([D, SLICE], F32, tag="fin", name="fin")
    all_groups = [[q * 4 + tl for tl in range(4)] for q in range(8)]
    for pi, (off, pw) in enumerate(PIECES):
        acc2 = psum.tile([D + 1, pw], F32, tag="acc", bufs=3,
                         name=f"acc2_{pi}")
        n_done = 0
        for grp in all_groups:
            gl = len(grp)
            vg = work.tile([P, gl * pw], F16, tag="vg", bufs=2,
                           name=f"vg2_{pi}_{grp[0]}")
            for k, t in enumerate(grp):
                u = u_of_t(t)
                nc.vector.tensor_scalar(
                    vg[:, k * pw:(k + 1) * pw], g2_row[:, off:off + pw],
                    rp_2[:, u:u + 1], e1_2[:, u:u + 1], ALU.mult, ALU.max)
            wg = work.tile([P, gl * pw], F16, tag="wg", bufs=4,
                           name=f"wg2_{pi}_{grp[0]}")
            nc.vector.tensor_tensor(wg[:], vg[:],
                                    adj_piece(pi, grp[0], gl), ALU.mult)
            for k, t in enumerate(grp):
                n_done += 1
                nc.tensor.matmul(
                    acc2[:], whs2[:, t * (D + 1):(t + 1) * (D + 1)],
                    wg[:, k * pw:(k + 1) * pw],
                    start=(t == 0), stop=(n_done == NT))
        lnr2 = work.tile([1, pw], F32, tag="lnr", bufs=2, name=f"ln_l2_{pi}")
        nc.scalar.activation(lnr2[:], acc2[D:D + 1, :], AF.Ln)
        rec2 = work.tile([1, pw], F32, tag="rec", bufs=2, name=f"rc_l2_{pi}")
        nc.scalar.activation(rec2[:], lnr2[:], AF.Exp, scale=-1.0)
        den2 = psum.tile([D, pw], F32, tag="bank", bufs=3,
                         name=f"den_l2_{pi}")
        nc.tensor.matmul(den2[:], ones_all[0:1, 0:D], rec2[0:1, :],
                         start=True, stop=True)
        num2 = work.tile([D, pw], F32, tag="num", bufs=3, name=f"num_l2_{pi}")
        nc.scalar.activation(num2[:], acc2[0:D, :], AF.Copy)
        x2 = work.tile([D, pw], F32, tag="xat", bufs=3, name=f"x_l2_{pi}")
        nc.vector.tensor_tensor(x2[:], num2[:], den2[:], ALU.mult)
        elu(f"l2_{pi}", x2[:], fin[:, off:off + pw], pw)
    nc.sync.dma_start(out=outT[:], in_=fin[:])


# ---------------------------------------------------------------------------
# host-side driver
# ---------------------------------------------------------------------------

def _prep_inputs(x, adj, W, a, Wo, ao):
    xT = x.T.astype(np.float16)                       # [F, N]
    xtr = np.ascontiguousarray(
        xT.reshape(NKF, P, N).transpose(1, 0, 2).reshape(P, NKF * N))
    wext = np.empty((F, H, E), np.float32)
    for h in range(H):
        a_src, a_dst = a[h, :D], a[h, D:]
        wext[:, h, 0:D] = W[h]
        wext[:, h, D] = W[h] @ a_src
        wext[:, h, D + 1] = W[h] @ a_dst
    wextr = np.ascontiguousarray(
        wext.reshape(NKF, P, H * E).transpose(1, 0, 2).reshape(P, -1)
    ).astype(np.float16)
    woAm = np.empty((D, H, E), np.float32)
    for h in range(H):
        Wo_h = Wo[h * D:(h + 1) * D]                  # [64 feat, 64 class]
        woAm[:, h, 0:D] = Wo_h
        woAm[:, h, D] = Wo_h @ ao[:D]
        woAm[:, h, D + 1] = Wo_h @ ao[D:]
    woAr = np.ascontiguousarray(woAm.reshape(D, H * E)).astype(np.float16)

    adjT = adj.T.astype(np.float16)                   # [j, i]
    in_maps = []
    for c in range(N_CORES):
        sl = slice(c * SLICE, (c + 1) * SLICE)
        asl = adjT[:, sl]
        blocks = []
        for off, pw in PIECES:
            blocks.append(
                asl[:, off:off + pw].reshape(NT, P, pw).transpose(1, 0, 2)
                .reshape(P, NT * pw))
        adjcm = np.ascontiguousarray(np.concatenate(blocks, axis=1))
        xslr = np.ascontiguousarray(
            xT[:, sl].reshape(NKF, P, SLICE).transpose(1, 0, 2)
            .reshape(P, NKF * SLICE))
        in_maps.append({
            "xtr": xtr, "xslr": xslr, "adjc": adjcm,
            "wextr": wextr, "woA": woAr,
        })
    return in_maps


def kernel(x, adj, W, a, Wo, ao, cfg):
    x = np.asarray(x, np.float32)
    adj = np.asarray(adj, np.float32)
    W = np.asarray(W, np.float32)
    a = np.asarray(a, np.float32)
    Wo = np.asarray(Wo, np.float32)
    ao = np.asarray(ao, np.float32)

    in_maps = _prep_inputs(x, adj, W, a, Wo, ao)
    if _CACHED.get("nc") is None:
        _CACHED["nc"] = build_kernel()
    res = run_bass_kernel_spmd(_CACHED["nc"], in_maps,
                               core_ids=list(range(N_CORES)))
    out = np.empty((N, D), np.float32)
    for c in range(N_CORES):
        out[c * SLICE:(c + 1) * SLICE, :] = res.results[c]["outT"].T
    return out


if __name__ == "__main__":
    import reference as ref_mod
    inputs = {k: np.asarray(v) for k, v in ref_mod.setup_inputs().items()}
    expected = np.asarray(ref_mod.reference(**ref_mod.setup_inputs()))
    got = kernel(**inputs)
    err = np.abs(got - expected).max() / np.abs(expected).max()
    print("rel err:", err)


# revision 56
# speedup vs baseline: 1.0068x; 1.0068x over previous
"""GAT (2-layer graph attention network) Trainium2 Bass kernel, 8-core SPMD.

Sharding (v2): every core computes ALL 8 layer-1 heads but only its own
512-column i-slice of the attention output (column-parallel), and the same
i-slice of layer 2. The adjacency slice [4096, 512] is loaded once per core
(two big DMAs) and reused by all 8 heads AND layer 2. Layer-1 -> layer-2
exchange is an AllGather of the per-core h @ Wo rows, split into a 384-row
and a 128-row piece so the first gather hides under the second piece's
attention loop.

Key math: exp(leaky_relu(s)) with s = f_src_i + f_dst_j factorizes as
e^{f_i} * max(g_i * r'_j, e1_j) with g_i = e^{(a-1) f_i},
r'_j = e^{a f_dst_j - C}, e1_j = e^{f_dst_j - C}. The e^{f_i} factor cancels
in the softmax, so the inner loop per (head, j-tile) is ONE
tensor_scalar (mult+max with two per-partition scalars, 4x DVE mode) and one
mask multiply (tensor_tensor, batched over 4 j-tiles), feeding a PE matmul
whose lhsT is the raw [Wh | 1] tile (ones column accumulates the softmax
denominator).

kernel(**inputs) takes full unsharded inputs, returns the full output.
"""

from contextlib import ExitStack

import numpy as np

import concourse.mybir as mybir
import concourse.tile as tile
from concourse import bacc
from concourse.bass_utils import run_bass_kernel_spmd
from concourse.masks import make_identity

# Steer every activation to the one ACT table set covering all functions this
# kernel uses (Exp, Copy, Identity) so no mid-kernel table reloads happen.
_orig_get_tables = bacc.get_activation_tables


def _pinned_tables(arch):
    tabs = _orig_get_tables(arch)
    if "natural_log_exp_and_others" in tabs:
        return {name: (funcs if name == "natural_log_exp_and_others" else set())
                for name, funcs in tabs.items()}
    return tabs


bacc.get_activation_tables = _pinned_tables

N = 4096
F = 512
D = 64          # per-head hidden == n classes
H = 8
P = 128
NT = N // P             # 32 j tiles
NKF = F // P            # 4 contraction tiles for x @ W
SLICE = N // 8          # 512 i columns per core
ALPHA = 0.2
AM1 = ALPHA - 1.0       # -0.8
C_DST = 1.0
PIECES = [(0, 512)]               # i-piece (offset, width) within the slice
N_CORES = 8
E = D + 2               # 66: [Wh | f_src | f_dst]

F32 = mybir.dt.float32
F16 = mybir.dt.float16

_CACHED = {}

AF = mybir.ActivationFunctionType
ALU = mybir.AluOpType


def build_kernel():
    nc = bacc.Bacc("TRN2", num_devices=N_CORES)

    xtr = nc.dram_tensor("xtr", [P, NKF * N], F16, kind="ExternalInput")
    xslr = nc.dram_tensor("xslr", [P, NKF * SLICE], F16, kind="ExternalInput")
    adjc = nc.dram_tensor("adjc", [P, NT * SLICE], F16, kind="ExternalInput")
    wextr = nc.dram_tensor("wextr", [P, NKF * H * E], F16,
                           kind="ExternalInput")
    woA = nc.dram_tensor("woA", [D, H * E], F16, kind="ExternalInput")
    outT = nc.dram_tensor("outT", [D, SLICE], F32, kind="ExternalOutput")

    with ExitStack() as ctx:
        tc = ctx.enter_context(tile.TileContext(nc))
        psum = ctx.enter_context(tc.tile_pool(name="psum", bufs=1, space="PSUM"))
        persist = ctx.enter_context(tc.tile_pool(name="persist", bufs=1))
        work = ctx.enter_context(tc.tile_pool(name="work", bufs=1))
        dram = ctx.enter_context(tc.tile_pool(name="dram", bufs=1, space="DRAM"))
        pools = {"psum": psum, "persist": persist, "work": work, "dram": dram}

        ident = persist.tile([P, P], F32, tag="ident")
        make_identity(nc, ident[:])
        ones_all = persist.tile([P, P], F32, tag="ones_all")
        nc.vector.memset(ones_all[:], 1.0)
        ones65 = persist.tile([D + 1, D], F32, tag="ones65")
        nc.vector.memset(ones65[:], 1.0)
        bias_d = persist.tile([P, 1], F32, tag="bias_d")
        nc.vector.memset(bias_d[:], -C_DST)
        _CACHED["bias_d"] = bias_d
        bias_p1 = persist.tile([P, 1], F32, tag="bias_p1")
        nc.vector.memset(bias_p1[:], 1.0)
        _CACHED["bias_p1"] = bias_p1
        _CACHED["ident"] = ident
        _CACHED["ones_all"] = ones_all
        _CACHED["ones65"] = ones65

        pools["tc"] = tc
        _emit(nc, pools, xtr, xslr, adjc, wextr, woA, outT)

    nc.compile()
    return nc


def _emit(nc, pools, xtr, xslr, adjc, wextr, woA, outT):
    psum, persist, work, dram = (pools["psum"], pools["persist"],
                                 pools["work"], pools["dram"])
    tc = pools["tc"]
    ident = _CACHED["ident"]
    ones_all = _CACHED["ones_all"]
    ones65 = _CACHED["ones65"]

    # ---- input DMAs (few, large) ----
    wextr_sb = persist.tile([P, NKF * H * E], F16, tag="wextr")
    nc.sync.dma_start(out=wextr_sb[:], in_=wextr[:])
    xt_sb = persist.tile([P, NKF * N], F16, tag="xt")
    NH = N // 2
    for half in range(2):
        nc.sync.dma_start(
            out=xt_sb[:].rearrange("p (kf n) -> p kf n", kf=NKF)[
                :, :, half * NH:(half + 1) * NH],
            in_=xtr[:].rearrange("p (kf n) -> p kf n", kf=NKF)[
                :, :, half * NH:(half + 1) * NH])
    xsl_sb = persist.tile([P, NKF * SLICE], F16, tag="xsl")
    nc.sync.dma_start(out=xsl_sb[:], in_=xslr[:])
    adj_sb = persist.tile([P, NT * SLICE], F16, tag="adj")
    QU = NT * SLICE // 4
    for q in range(4):
        nc.sync.dma_start(out=adj_sb[:, q * QU:(q + 1) * QU],
                          in_=adjc[:, q * QU:(q + 1) * QU])
    woA_sb = persist.tile([D, H * E], F16, tag="woA")
    nc.sync.dma_start(out=woA_sb[:], in_=woA[:])

    # ---- per-head prep: whs = [Wh | 1] fp16, e1 = exp(f_dst - C),
    #      r' = exp(a*f_dst - C), g_row = exp((a-1) f_src) broadcast ----
    whs, e1_sb, rp_sb, g_row = [], [], [], []
    for h in range(H):
        wb = persist.tile([P, NT * (D + 1)], F16, tag=f"whs{h}",
                          name=f"whs_{h}")
        nc.vector.memset(
            wb[:].rearrange("p (t c) -> p t c", t=NT)[:, :, D:D + 1], 1.0)
        e1 = persist.tile([P, NT], F32, tag=f"e1_{h}", name=f"e1_{h}")
        rp = persist.tile([P, NT], F32, tag=f"rp_{h}", name=f"rp_{h}")
        wb3 = wb[:].rearrange("p (t c) -> p t c", t=NT)
        fcol = work.tile([P, NT], F32, tag="fcol", bufs=2, name=f"fcol_{h}")
        fc3 = fcol[:].rearrange("p (k one) -> p k one", one=1)
        for grp in range(NT // 4):
            wh_ps = psum.tile([P, 4 * E], F32, tag="bank", bufs=3,
                              name=f"whps_{h}_{grp}")
            for k in range(4):
                t = grp * 4 + k
                for kf in range(NKF):
                    nc.tensor.matmul(
                        wh_ps[:, k * E:(k + 1) * E],
                        xt_sb[:, kf * N + t * P:kf * N + (t + 1) * P],
                        wextr_sb[:, (kf * H + h) * E:(kf * H + h + 1) * E],
                        start=(kf == 0), stop=(kf == NKF - 1))
            ps3 = wh_ps[:].rearrange("p (k c) -> p k c", k=4)
            # stage the f_dst column; exp per half-head so attention on the
            # first tiles can start before the whole head's prep finishes
            nc.scalar.activation(fc3[:, grp * 4:(grp + 1) * 4, :],
                                 ps3[:, :, D + 1:D + 2], AF.Copy)
            nc.scalar.activation(wb3[:, grp * 4:(grp + 1) * 4, 0:D],
                                 ps3[:, :, 0:D], AF.Copy)
            if grp == 3 or grp == 7:
                hsl = slice(0, 16) if grp == 3 else slice(16, 32)
                nc.scalar.activation(e1[:, hsl], fcol[:, hsl], AF.Exp,
                                     bias=_CACHED["bias_d"][:])
                nc.scalar.activation(rp[:, hsl], fcol[:, hsl], AF.Exp,
                                     bias=_CACHED["bias_d"][:], scale=ALPHA)
        whs.append(wb)
        e1_sb.append(e1)
        rp_sb.append(rp)

        fs_ps = psum.tile([1, SLICE], F32, tag="bank", bufs=3,
                          name=f"fsps_{h}")
        for kf in range(NKF):
            nc.tensor.matmul(
                fs_ps[:],
                wextr_sb[:, (kf * H + h) * E + D:(kf * H + h) * E + D + 1],
                xsl_sb[:, kf * SLICE:(kf + 1) * SLICE],
                start=(kf == 0), stop=(kf == NKF - 1))
        fs_sb = work.tile([1, SLICE], F32, tag="fs_sb", bufs=2,
                          name=f"fssb_{h}")
        nc.scalar.activation(fs_sb[:], fs_ps[:], AF.Copy)
        g_ps = psum.tile([P, SLICE], F32, tag="bank", bufs=3, name=f"gps_{h}")
        for ch in range(SLICE // P):
            nc.tensor.matmul(g_ps[:, ch * P:(ch + 1) * P],
                             ones_all[0:1, 0:P],
                             fs_sb[0:1, ch * P:(ch + 1) * P],
                             start=True, stop=True)
        gr = persist.tile([P, SLICE], F16, tag=f"g_{h}", name=f"g_{h}")
        nc.scalar.activation(gr[:], g_ps[:], AF.Exp, scale=AM1)
        g_row.append(gr)

    # ---- layer-1 attention + (pipelined) norm/ELU/p2, two i-pieces ----
    p2AB_ps = psum.tile([P, 4 * E], F32, tag="p2AB", name="p2AB")
    p2A_ps = p2AB_ps[:, 0:4 * E]
    cc_inA = dram.tile([4 * P, E], F16, tag="cc_inA", name="cc_inA")
    cc_fullA = dram.tile([N_CORES * 4 * P, E], F16, tag="cc_fullA",
                         addr_space="Shared", name="cc_fullA")

    ADJ_B0 = NT * PIECES[0][1]     # start col of the B block

    def adj_piece(pi, t, ntiles=1):
        pw = PIECES[pi][1]
        base = 0 if pi == 0 else ADJ_B0
        return adj_sb[:, base + t * pw:base + (t + ntiles) * pw]

    def attention(tag, pi, pw, g_ap, rp_ap, e1_ap, whs_ap, acc, hh=0):
        off = PIECES[pi][0]
        for grp in range(NT // 4):
            # offload a fraction of whole groups (TSPs + mask TT) to the
            # otherwise-idle GpSimd engine
            on_pool = ((hh * 8 + grp) % 5) == 4
            eng = nc.gpsimd if on_pool else nc.vector
            vg = work.tile([P, 4 * pw], F16, tag="vg", bufs=3,
                           name=f"vg_{tag}_{grp}")
            for k in range(4):
                t = grp * 4 + k
                eng.tensor_scalar(
                    vg[:, k * pw:(k + 1) * pw], g_ap[:, off:off + pw],
                    rp_ap[:, t:t + 1], e1_ap[:, t:t + 1], ALU.mult, ALU.max)
            wg = work.tile([P, 4 * pw], F16, tag="wg", bufs=4,
                           name=f"wg_{tag}_{grp}")
            eng.tensor_tensor(wg[:], vg[:], adj_piece(pi, grp * 4, 4),
                              ALU.mult)
            for k in range(4):
                t = grp * 4 + k
                nc.tensor.matmul(
                    acc[:], whs_ap[:, t * (D + 1):(t + 1) * (D + 1)],
                    wg[:, k * pw:(k + 1) * pw],
                    start=(t == 0), stop=(t == NT - 1))

    def elu(tag, x_ap, out_ap, pw):
        # elu(x) = max(x,0) - relu(1 - exp(x))
        e = work.tile([D, pw], F32, tag="elu_e", bufs=2, name=f"ele_{tag}")
        nc.scalar.activation(e[:], x_ap, AF.Exp)
        t1 = work.tile([D, pw], F32, tag="elu_t", bufs=2, name=f"elt_{tag}")
        nc.scalar.activation(t1[:], e[:], AF.Relu, scale=-1.0,
                             bias=_CACHED["bias_p1"][0:D, :])
        nc.vector.scalar_tensor_tensor(out_ap, x_ap, 0.0, t1[:],
                                       ALU.max, ALU.subtract)

    def norm_elu_p2(tag, acc, pw, h, fs2_ps):
        lnr = work.tile([1, pw], F32, tag="lnr", bufs=2, name=f"ln_{tag}")
        nc.scalar.activation(lnr[:], acc[D:D + 1, :], AF.Ln)
        rec = work.tile([1, pw], F32, tag="rec", bufs=2, name=f"rc_{tag}")
        nc.scalar.activation(rec[:], lnr[:], AF.Exp, scale=-1.0)
        den = psum.tile([D, pw], F32, tag="bank", bufs=3, name=f"den_{tag}")
        nc.tensor.matmul(den[:], ones_all[0:1, 0:D], rec[0:1, :],
                         start=True, stop=True)
        num = work.tile([D, pw], F32, tag="num", bufs=3, name=f"num_{tag}")
        nc.scalar.activation(num[:], acc[0:D, :], AF.Copy)
        x = work.tile([D, pw], F32, tag="xat", bufs=3, name=f"x_{tag}")
        nc.vector.tensor_tensor(x[:], num[:], den[:], ALU.mult)
        o_sb = work.tile([D, pw], F16, tag="osb", bufs=10, name=f"o_{tag}")
        elu(tag, x[:], o_sb[:], pw)
        # f_src2 row accumulates in its own bank; one open group there is
        # safe. The p2 chunk matmuls are emitted after the head loop so
        # each bank has at most one open accumulation group at a time.
        nc.tensor.matmul(
            fs2_ps, woA_sb[:, h * E + D:h * E + D + 1], o_sb[:],
            start=(h == 0), stop=(h == H - 1))
        return o_sb

    p2_sb = {}
    fs2_sb = work.tile([1, SLICE], F32, tag="fs2_sb", name="fs2_sb")
    fs2all_ps = psum.tile([1, SLICE], F32, tag="fs2", name="fs2")
    for pi, (off, pw) in enumerate(PIECES):
        nch = pw // P
        p2_ps = p2A_ps

        fs2_ps = fs2all_ps[0:1, off:off + pw]
        pending = None
        o_list = []
        for h in range(H):
            acc = psum.tile([D + 1, pw], F32, tag="acc", bufs=3,
                            name=f"acc_{pi}_{h}")
            attention(f"l1_{pi}_{h}", pi, pw, g_row[h][:], rp_sb[h][:],
                      e1_sb[h][:], whs[h][:], acc, hh=h)
            if pending is not None:
                o_list.append(norm_elu_p2(
                    f"l1_{pi}_{pending[0]}", pending[1], pw, pending[0],
                    fs2_ps))
            pending = (h, acc)
        with tc.high_priority():
            o_list.append(norm_elu_p2(f"l1_{pi}_{pending[0]}", pending[1],
                                      pw, pending[0], fs2_ps))
            for ch in range(nch):
                for h in range(H):
                    nc.tensor.matmul(
                        p2_ps[:, ch * E:(ch + 1) * E],
                        o_list[h][:, ch * P:(ch + 1) * P],
                        woA_sb[:, h * E:(h + 1) * E],
                        start=(h == 0), stop=(h == H - 1))
            nc.scalar.activation(fs2_sb[0:1, off:off + pw], fs2_ps, AF.Copy)
            ps = persist.tile([P, nch * E], F16, tag=f"p2sb_{pi}",
                              name=f"p2sb_{pi}")
            nc.scalar.activation(ps[:], p2_ps, AF.Copy)
            p2_sb[pi] = ps
            cc_in = cc_inA
            nc.sync.dma_start(
                out=cc_in[:].rearrange("(c p) d -> p c d", p=P),
                in_=ps[:].rearrange("p (c d) -> p c d", c=nch))

    nc.gpsimd.collective_compute(
        "AllGather", ALU.bypass, ins=[cc_inA[:]], outs=[cc_fullA[:]],
        replica_groups=[list(range(N_CORES))])

    # ---- g2_row from this core's own f_src2 row ----
    g2_ps = psum.tile([P, SLICE], F32, tag="bank", bufs=3, name="g2ps")
    for ch in range(SLICE // P):
        nc.tensor.matmul(g2_ps[:, ch * P:(ch + 1) * P],
                         ones_all[0:1, 0:P], fs2_sb[0:1, ch * P:(ch + 1) * P],
                         start=True, stop=True)
    g2_row = persist.tile([P, SLICE], F16, tag="g2", name="g2")
    nc.scalar.activation(g2_row[:], g2_ps[:], AF.Exp, scale=AM1)

    # ---- layer-2 prep from gathered rows ----
    whs2 = persist.tile([P, NT * (D + 1)], F16, tag="whs2", name="whs2")
    nc.vector.memset(
        whs2[:].rearrange("p (t c) -> p t c", t=NT)[:, :, D:D + 1], 1.0)
    e1_2 = persist.tile([P, NT], F32, tag="e1_2", name="e1_2")
    rp_2 = persist.tile([P, NT], F32, tag="rp_2", name="rp_2")

    ccA_sb = persist.tile([P, NT * E], F16, tag="ccA", name="ccA")
    NU = NT // 2
    for uh in range(2):
        nc.sync.dma_start(
            out=ccA_sb[:].rearrange("p (u d) -> p u d", u=NT)[
                :, uh * NU:(uh + 1) * NU, :],
            in_=cc_fullA[:].rearrange("(u p) d -> p u d", p=P)[
                :, uh * NU:(uh + 1) * NU, :])

    w23 = whs2[:].rearrange("p (t c) -> p t c", t=NT)

    def u_of_t(t):
        return t

    def l2_prep(cc_sb, n_u, u0, g_of_u):
        cc3 = cc_sb[:].rearrange("p (u c) -> p u c", u=NT)[
            :, u0:u0 + n_u, :]
        fc2 = work.tile([P, n_u], F32, tag="fc2", bufs=2, name=f"fc2_{u0}")
        nc.scalar.activation(
            fc2[:].rearrange("p (u one) -> p u one", one=1),
            cc3[:, :, D + 1:D + 2], AF.Copy)
        nc.scalar.activation(e1_2[:, u0:u0 + n_u], fc2[:], AF.Exp,
                             bias=_CACHED["bias_d"][:])
        nc.scalar.activation(rp_2[:, u0:u0 + n_u], fc2[:], AF.Exp,
                             bias=_CACHED["bias_d"][:], scale=ALPHA)
        for uu in range(n_u):
            u = u0 + uu
            if u % 2 == 0:
                nc.scalar.activation(w23[:, g_of_u(u), 0:D],
                                     cc3[:, uu, 0:D], AF.Copy)
            else:
                nc.gpsimd.tensor_copy(w23[:, g_of_u(u), 0:D],
                                      cc3[:, uu, 0:D])

    l2_prep(ccA_sb, NT // 2, 0, lambda u: u)
    l2_prep(ccA_sb, NT // 2, NT // 2, lambda u: u)

    # ---- layer-2 attention, same two i-pieces as layer 1 ----
    fin = persist.tile([D, SLICE], F32, tag="fin", name="fin")
    all_groups = [[q * 4 + tl for tl in range(4)] for q in range(8)]
    for pi, (off, pw) in enumerate(PIECES):
        acc2 = psum.tile([D + 1, pw], F32, tag="acc", bufs=3,
                         name=f"acc2_{pi}")
        n_done = 0
        for grp in all_groups:
            gl = len(grp)
            vg = work.tile([P, gl * pw], F16, tag="vg", bufs=2,
                           name=f"vg2_{pi}_{grp[0]}")
            for k, t in enumerate(grp):
                u = u_of_t(t)
                nc.vector.tensor_scalar(
                    vg[:, k * pw:(k + 1) * pw], g2_row[:, off:off + pw],
                    rp_2[:, u:u + 1], e1_2[:, u:u + 1], ALU.mult, ALU.max)
            wg = work.tile([P, gl * pw], F16, tag="wg", bufs=4,
                           name=f"wg2_{pi}_{grp[0]}")
            nc.vector.tensor_tensor(wg[:], vg[:],
                                    adj_piece(pi, grp[0], gl), ALU.mult)
            for k, t in enumerate(grp):
                n_done += 1
                nc.tensor.matmul(
                    acc2[:], whs2[:, t * (D + 1):(t + 1) * (D + 1)],
                    wg[:, k * pw:(k + 1) * pw],
                    start=(t == 0), stop=(n_done == NT))
        lnr2 = work.tile([1, pw], F32, tag="lnr", bufs=2, name=f"ln_l2_{pi}")
        nc.scalar.activation(lnr2[:], acc2[D:D + 1, :], AF.Ln)
        rec2 = work.tile([1, pw], F32, tag="rec", bufs=2, name=f"rc_l2_{pi}")
        nc.scalar.activation(rec2[:], lnr2[:], AF.Exp, scale=-1.0)
        den2 = psum.tile([D, pw], F32, tag="bank", bufs=3,
                         name=f"den_l2_{pi}")
        nc.tensor.matmul(den2[:], ones_all[0:1, 0:D], rec2[0:1, :],
                         start=True, stop=True)
        num2 = work.tile([D, pw], F32, tag="num", bufs=3, name=f"num_l2_{pi}")
        nc.scalar.activation(num2[:], acc2[0:D, :], AF.Copy)
        x2 = work.tile([D, pw], F32, tag="xat", bufs=3, name=f"x_l2_{pi}")
        nc.vector.tensor_tensor(x2[:], num2[:], den2[:], ALU.mult)
        elu(f"l2_{pi}", x2[:], fin[:, off:off + pw], pw)
    nc.sync.dma_start(out=outT[:], in_=fin[:])


# ---------------------------------------------------------------------------
# host-side driver
# ---------------------------------------------------------------------------

def _prep_inputs(x, adj, W, a, Wo, ao):
    xT = x.T.astype(np.float16)                       # [F, N]
    xtr = np.ascontiguousarray(
        xT.reshape(NKF, P, N).transpose(1, 0, 2).reshape(P, NKF * N))
    wext = np.empty((F, H, E), np.float32)
    for h in range(H):
        a_src, a_dst = a[h, :D], a[h, D:]
        wext[:, h, 0:D] = W[h]
        wext[:, h, D] = W[h] @ a_src
        wext[:, h, D + 1] = W[h] @ a_dst
    wextr = np.ascontiguousarray(
        wext.reshape(NKF, P, H * E).transpose(1, 0, 2).reshape(P, -1)
    ).astype(np.float16)
    woAm = np.empty((D, H, E), np.float32)
    for h in range(H):
        Wo_h = Wo[h * D:(h + 1) * D]                  # [64 feat, 64 class]
        woAm[:, h, 0:D] = Wo_h
        woAm[:, h, D] = Wo_h @ ao[:D]
        woAm[:, h, D + 1] = Wo_h @ ao[D:]
    woAr = np.ascontiguousarray(woAm.reshape(D, H * E)).astype(np.float16)

    adjT = adj.T.astype(np.float16)                   # [j, i]
    in_maps = []
    for c in range(N_CORES):
        sl = slice(c * SLICE, (c + 1) * SLICE)
        asl = adjT[:, sl]
        blocks = []
        for off, pw in PIECES:
            blocks.append(
                asl[:, off:off + pw].reshape(NT, P, pw).transpose(1, 0, 2)
                .reshape(P, NT * pw))
        adjcm = np.ascontiguousarray(np.concatenate(blocks, axis=1))
        xslr = np.ascontiguousarray(
            xT[:, sl].reshape(NKF, P, SLICE).transpose(1, 0, 2)
            .reshape(P, NKF * SLICE))
        in_maps.append({
            "xtr": xtr, "xslr": xslr, "adjc": adjcm,
            "wextr": wextr, "woA": woAr,
        })
    return in_maps


def kernel(x, adj, W, a, Wo, ao, cfg):
    x = np.asarray(x, np.float32)
    adj = np.asarray(adj, np.float32)
    W = np.asarray(W, np.float32)
    a = np.asarray(a, np.float32)
    Wo = np.asarray(Wo, np.float32)
    ao = np.asarray(ao, np.float32)

    in_maps = _prep_inputs(x, adj, W, a, Wo, ao)
    if _CACHED.get("nc") is None:
        _CACHED["nc"] = build_kernel()
    res = run_bass_kernel_spmd(_CACHED["nc"], in_maps,
                               core_ids=list(range(N_CORES)))
    out = np.empty((N, D), np.float32)
    for c in range(N_CORES):
        out[c * SLICE:(c + 1) * SLICE, :] = res.results[c]["outT"].T
    return out


if __name__ == "__main__":
    import reference as ref_mod
    inputs = {k: np.asarray(v) for k, v in ref_mod.setup_inputs().items()}
    expected = np.asarray(ref_mod.reference(**ref_mod.setup_inputs()))
    got = kernel(**inputs)
    err = np.abs(got - expected).max() / np.abs(expected).max()
    print("rel err:", err)
